# revision 40
# baseline (speedup 1.0000x reference)
"""Causal multi-head self-attention block (B=8, T=1024, C=768, H=12) on 8 TRN2
NeuronCores, data-parallel over the batch dimension: core b computes batch b
end-to-end (no collectives).

Layout strategy (per core):
  - host passes x[b] pre-transposed: xT [C=768, T=1024]
  - qT,kT are produced channel-major ([ch, tok]) by using w_attn slices as the
    stationary matmul operand; v is produced token-major ([tok, ch]) by using
    xT slices as stationary.  Both orientations consume the same xT, so the
    kernel needs no on-chip transposes at all.
  - attention scores are computed transposed, attT[k, q] = kT_h.T-free matmul,
    softmax'd as exp(score) without max subtraction (logits here are ~N(0,0.3),
    so exp is safe in fp32), causal-masked via memset + one triangular tile.
  - the softmax denominator is obtained for free by appending a ones column to
    the stationary v operand (65-wide lhsT): row 64 of the AV psum is the sum.
  - y accumulates channel-major (yT), which feeds the output projection with
    w_proj in its native layout; biases are folded in as K=1 matmuls.
"""

from contextlib import ExitStack

import ml_dtypes
import numpy as np

import concourse.bass as bass
import concourse.tile as tile
from concourse import bacc, mybir

N_CORES = 8
B, T, C = 8, 1024, 768
H, HD = 12, 64
C3 = 3 * C
DT = mybir.dt.float32
BF = mybir.dt.bfloat16
AF = mybir.ActivationFunctionType
P = 128
KC = C // P            # 6 k-tiles over the embedding dim
NSUB = (2 * C) // P    # 12 channel blocks covering q and k
TB = T // P            # 8 token blocks
QS = T // 512          # 2 query slices of 512


def _emit(tc: tile.TileContext, io: dict, with_vbias: bool = True, with_pbias: bool = True) -> None:
    nc = tc.nc
    xT_d, wqk_d, wv_d, bqk_d, bv_d, tri_d, wp_d, bp_d, out_d = (
        io["xT"], io["wqk"], io["wv2"], io["bqk"], io["bv"], io["tri"],
        io["wp"], io["bp"], io["out"],
    )

    stack = ExitStack()
    const = stack.enter_context(tc.tile_pool(name="const", bufs=1))
    persist = stack.enter_context(tc.tile_pool(name="persist", bufs=1))

    # ---- constants (tiles; DMAs for non-critical ones are emitted later
    # so the startup burst only carries what the first matmuls need) -------
    tri = const.tile([P, P], BF, tag="tri")          # tri[i,j] = j >= i
    bqk = const.tile([P, NSUB], DT, tag="bqk")       # per-channel qk bias
    nc.sync.dma_start(bqk[:], bqk_d[:, :])
    bv = const.tile([1, C], BF, tag="bv")            # v bias row
    bp = const.tile([1, C], BF, tag="bp")            # proj bias row
    ones = const.tile([1, T], BF, tag="ones")
    nc.vector.memset(ones[:], 1.0)

    # ---- persistent activations -----------------------------------------
    # qkT[s]: channel-major q/k; s 0-5 -> q channels, 6-11 -> k channels
    qkT = [persist.tile([P, T], BF, tag=f"qkT{s}", name=f"qkT{s}") for s in range(NSUB)]
    # v3[tb]: token-major v for token block tb, one 65-wide group per head
    # (64 channels + a ones column used to accumulate softmax denominators)
    v3 = [persist.tile([P, H, HD + 1], BF, tag=f"v{tb}", name=f"v{tb}") for tb in range(TB)]
    # yT[qs][kc]: channel-major attention output, split per q-slice so the
    # qs=0 projection never serializes against qs=1 writes (tile-granular deps)
    yT = [[persist.tile([P, 512], BF, tag=f"yT{q}_{k}", name=f"yT{q}_{k}")
           for k in range(KC)] for q in range(QS)]
    # w_proj, native layout (loaded later, after the hot startup DMAs)
    wp = [persist.tile([P, C], BF, tag=f"wp{k}", name=f"wp{k}") for k in range(KC)]

    # =====================================================================
    # Phase 1: qkv projection
    # =====================================================================
    p1stack = ExitStack()
    ph1 = p1stack.enter_context(tc.tile_pool(name="ph1", bufs=1))
    ph1s = p1stack.enter_context(tc.tile_pool(name="ph1s", bufs=3))
    psum1 = p1stack.enter_context(tc.tile_pool(name="psum1", bufs=2, space="PSUM"))

    # xT split into 512-column halves: the first accumulation chain (i=0)
    # only needs the first-half tiles, so the PE starts ~2x sooner
    xT = [[ph1.tile([P, 512], BF, tag=f"xT{k}_{i}", name=f"xT{k}_{i}")
           for i in range(2)] for k in range(KC)]
    wsubs = [ph1s.tile([P, KC, P], BF, tag="wsub", name=f"wsub{s}", bufs=NSUB)
             for s in range(NSUB)]
    # DMA order = first-consumer order
    nc.sync.dma_start(xT[0][0][:], xT_d[0:P, 0:512])
    nc.gpsimd.dma_start(wsubs[0][:], wqk_d[0])
    for k in range(1, KC):
        nc.sync.dma_start(xT[k][0][:], xT_d[k * P:(k + 1) * P, 0:512])
    for k in range(KC):
        nc.sync.dma_start(xT[k][1][:], xT_d[k * P:(k + 1) * P, 512:1024])
    for s in range(1, NSUB):
        nc.gpsimd.dma_start(wsubs[s][:], wqk_d[s])
    # v-part of w_attn (host-transposed so the load is fully contiguous)
    wv = [ph1.tile([P, KC, 384], BF, tag=f"wv{c}", name=f"wv{c}") for c in range(2)]
    for c in range(2):
        nc.sync.dma_start(wv[c][:], wv_d[c])

    # ones columns of v3 (softmax denominator accumulators)
    for tb in range(TB):
        nc.vector.memset(v3[tb][:, :, HD:HD + 1], 1.0)

    # --- qT / kT: channel-major, w_attn slice stationary -----------------
    for s in range(NSUB):
        wsub = wsubs[s]
        ps = [psum1.tile([P, 512], DT, tag=f"qk{i}", name=f"qk{i}") for i in range(2)]
        for k in range(KC):
            for i in range(2):
                nc.tensor.matmul(
                    ps[i][:], lhsT=wsub[:, k, :], rhs=xT[k][i][:],
                    start=(k == 0), stop=(k == KC - 1),
                )
        # bias add (+ 1/sqrt(hd) folded into q) during psum->sbuf copy,
        # alternating DVE/ACT so neither engine becomes the choke point
        scl = 0.125 if s < KC else 1.0
        for i in range(2):
            dst = qkT[s][:, i * 512:(i + 1) * 512]
            if (2 * s + i) % 2 == 0:
                nc.vector.tensor_scalar(
                    dst, ps[i][:], scl, bqk[:, s:s + 1],
                    mybir.AluOpType.mult, mybir.AluOpType.add,
                )
            else:
                nc.scalar.activation(
                    dst, ps[i][:], AF.Identity, bias=bqk[:, s:s + 1], scale=scl
                )

    # remaining loads, off the startup critical path
    nc.sync.dma_start(tri[:], tri_d[:, :])
    nc.sync.dma_start(bv[:], bv_d[:, :])
    nc.sync.dma_start(bp[:], bp_d[:, :])
    for k in range(KC):
        nc.sync.dma_start(wp[k][:], wp_d[k * P:(k + 1) * P, :])

    # --- v: token-major, xT slice stationary -----------------------------
    for tb in range(TB):
        for c in range(2):
            pv = psum1.tile([P, 384], DT, tag="vps")
            for k in range(KC):
                nc.tensor.matmul(
                    pv[:], lhsT=xT[k][tb // 4][:, (tb % 4) * P:(tb % 4 + 1) * P],
                    rhs=wv[c][:, k, :], start=(k == 0),
                    stop=(not with_vbias and k == KC - 1),
                )
            if with_vbias:
                nc.tensor.matmul(  # + bias via K=1 ones row
                    pv[:], lhsT=ones[:, tb * P:(tb + 1) * P],
                    rhs=bv[:, c * 384:(c + 1) * 384], start=False, stop=True,
                )
            nc.vector.tensor_copy(
                v3[tb][:, c * 6:(c + 1) * 6, 0:HD],
                pv[:].rearrange("p (h f) -> p h f", h=6),
            )

    p1stack.close()

    # =====================================================================
    # Phase 2: attention (scores transposed: [k, q]) + output projection
    # =====================================================================
    p2stack = ExitStack()
    ph2 = p2stack.enter_context(tc.tile_pool(name="ph2", bufs=2))
    psum2_cm = tc.tile_pool(name="psum2", bufs=1, space="PSUM")
    psum2 = psum2_cm.__enter__()
    dram2 = p2stack.enter_context(tc.tile_pool(name="dram2", bufs=2, space="DRAM"))

    def emit_scores_pair(qs, j):
        """score+exp+mask for heads (2j, 2j+1) over all causal k-blocks.

        The two heads live at partition bases 0 and 64 of the same qkT tile,
        so their K=64 matmuls land in different PE row groups and run
        concurrently (tile_position auto-derived from base_partition)."""
        nkb = 4 * (qs + 1)
        ets = ([], [])
        for kb in range(nkb):
            # columns < d are causally dead: matmul/exp/av all skip them
            d = max(kb * P - qs * 512, 0)
            pt = psum2.tile([P, 1024], DT, tag="sps", name="sps", bufs=3)
            for half in range(2):
                po = half * HD
                nc.tensor.matmul(
                    pt[:, half * 512 + d:(half + 1) * 512],
                    lhsT=qkT[KC + j][po:po + HD, kb * P:(kb + 1) * P],
                    rhs=qkT[j][po:po + HD, qs * 512 + d:(qs + 1) * 512],
                    start=True, stop=True,
                )
            # one ACT op exps both heads' valid columns (3D strided AP)
            e = ph2.tile([P, 1024], BF, tag=f"E{kb}", name=f"E{kb}")
            er = e[:].rearrange("p (a f) -> p a f", a=2)[:, :, d:512]
            pr = pt[:].rearrange("p (a f) -> p a f", a=2)[:, :, d:512]
            nc.scalar.activation(er, pr, AF.Exp)
            if kb * P - qs * 512 >= 0:  # diagonal block: triangular mask
                ed = e[:].rearrange("p (a f) -> p a f", a=2)[:, :, d:d + P]
                nc.vector.tensor_tensor(
                    ed, ed, tri[:, None, :].to_broadcast((P, 2, P)),
                    mybir.AluOpType.mult,
                )
            ets[0].append(e)
            ets[1].append(e)
        return ets

    def emit_av_raw(qs, h, etiles, sums_t):
        """unnormalized yT_h[:, qs] = v_ext.T @ E; the fused ones column puts
        the softmax denominator in psum row 64, gathered into sums_t[h]."""
        nkb = len(etiles)
        po, so = (h % 2) * HD, h // 2
        eo = (h % 2) * 512  # this head's half of the fused E tile
        psy = psum2.tile([HD + 1, 512], DT, tag="yps", name="yps", bufs=2)
        for kb in range(nkb):
            d = max(kb * P - qs * 512, 0)
            nc.tensor.matmul(
                psy[:, d:512], lhsT=v3[kb][:, h, :],
                rhs=etiles[kb][:, eo + d:eo + 512],
                start=(kb == 0), stop=(kb == nkb - 1),
            )
        # DVE output bases are limited to 0/32/64/96, so route the sums row
        # into partition h of the gather tile through DRAM (sums_t is DRAM)
        stg = ph2.tile([1, 512], DT, tag="sstage", name="sstage", bufs=3)
        nc.vector.tensor_copy(stg[:], psy[HD:HD + 1, :])
        nc.gpsimd.dma_start(sums_t[h:h + 1, :], stg[:])  # SBUF->SBUF, any partition
        nc.vector.tensor_copy(yT[qs][so][po:po + HD, :], psy[0:HD, :])

    def emit_finish(qs, sums_t):
        """batched reciprocal of all 12 denominators, partition-broadcast via
        a DRAM bounce (SBUF APs need nonzero partition step), normalize yT."""
        rinv = ph2.tile([H, 512], DT, tag="rinv", name="rinv")
        nc.vector.reciprocal(rinv[:], sums_t[:H, :])
        rd = dram2.tile([H, 512], DT, tag="rd", name="rd")
        nc.gpsimd.dma_start(rd[:], rinv[:])
        for j in range(KC):
            # heads 2j/2j+1 -> partitions 0:64/64:128 so every tensor_mul has
            # all SBUF operands at the same start partition (walrus rule)
            rb = ph2.tile([P, 512], DT, tag="rb", name="rb", bufs=3)
            for half in range(2):
                nc.gpsimd.dma_start(
                    rb[half * HD:(half + 1) * HD, :],
                    rd[2 * j + half:2 * j + half + 1, :].to_broadcast((HD, 512)),
                )
            for half in range(2):
                po = half * HD
                ys = yT[qs][j][po:po + HD, :]
                nc.vector.tensor_mul(ys, ys, rb[po:po + HD, :])

    def emit_proj(qs, psum_o):
        for tb in range(4 * qs, 4 * qs + 4):
            for c in range(2):
                pso = psum_o.tile([P, 384], DT, tag="ops", name="ops", bufs=4)
                for k in range(KC):
                    nc.tensor.matmul(
                        pso[:],
                        lhsT=yT[qs][k][:, (tb % 4) * P:(tb % 4 + 1) * P],
                        rhs=wp[k][:, c * 384:(c + 1) * 384],
                        start=(k == 0),
                        stop=(not with_pbias and k == KC - 1),
                    )
                if with_pbias:
                    nc.tensor.matmul(
                        pso[:], lhsT=ones[:, tb * P:(tb + 1) * P],
                        rhs=bp[:, c * 384:(c + 1) * 384], start=False, stop=True,
                    )
                osb = ph2.tile([P, 384], DT, tag="osb", name="osb", bufs=3)
                nc.scalar.activation(osb[:], pso[:], AF.Copy)
                nc.sync.dma_start(
                    out_d[tb * P:(tb + 1) * P, c * 384:(c + 1) * 384], osb[:]
                )

    # Software pipeline at head-pair granularity: scores of pair p+1 are
    # emitted before av of pair p so the PE never waits on the ACT/DVE
    # exp+mask chain; the previous q-slice's normalize+proj is emitted two
    # pairs into the next q-slice so its PE work overlaps attention.
    sums_tiles = {}
    prev = None   # (qs, j, ets)
    fin = None    # q-slice whose normalize+proj is pending
    for qs in range(QS):
        sums_tiles[qs] = ph2.tile([H, 512], DT, tag="sums", name="sums")
        for j in range(KC):
            ets = emit_scores_pair(qs, j)
            if prev is not None:
                pq, pj, pets = prev
                for half in range(2):
                    emit_av_raw(pq, 2 * pj + half, pets[half], sums_tiles[pq])
            prev = (qs, j, ets)
            if fin is not None and j == 3:
                emit_finish(fin, sums_tiles[fin])
                fin = None
        fin = qs
    pq, pj, pets = prev
    for half in range(2):
        emit_av_raw(pq, 2 * pj + half, pets[half], sums_tiles[pq])
    # attention psum pool closes; proj(0) keeps the PE busy while
    # finish(1)'s reciprocal/broadcast chain resolves on DVE/DMA
    psum2_cm.__exit__(None, None, None)
    psum3 = p2stack.enter_context(tc.tile_pool(name="psum3", bufs=1, space="PSUM"))
    emit_finish(1, sums_tiles[1])
    emit_proj(0, psum3)
    emit_proj(1, psum3)

    p2stack.close()
    stack.close()


def build_program(with_vbias: bool = True, with_pbias: bool = True) -> tuple[bass.Bass, dict]:
    nc = bacc.Bacc("TRN2", debug=False)
    io = {
        "xT": nc.dram_tensor("xT", [C, T], BF, kind="ExternalInput"),
        "wqk": nc.dram_tensor("wqk", [NSUB, P, KC, P], BF, kind="ExternalInput"),
        "wv2": nc.dram_tensor("wv2", [2, P, KC, 384], BF, kind="ExternalInput"),
        "bqk": nc.dram_tensor("bqk", [P, NSUB], DT, kind="ExternalInput"),
        "bv": nc.dram_tensor("bv", [1, C], BF, kind="ExternalInput"),
        "tri": nc.dram_tensor("tri", [P, P], BF, kind="ExternalInput"),
        "wp": nc.dram_tensor("wp", [C, C], BF, kind="ExternalInput"),
        "bp": nc.dram_tensor("bp", [1, C], BF, kind="ExternalInput"),
        "out": nc.dram_tensor("out", [T, C], DT, kind="ExternalOutput"),
    }
    with tile.TileContext(nc) as tc:
        _emit(tc, io, with_vbias=with_vbias, with_pbias=with_pbias)
    nc.compile()
    return nc, io


_CACHED = {}


def make_in_maps(x, w_attn, b_attn, w_proj, b_proj):
    x = np.asarray(x, np.float32)
    w_attn = np.asarray(w_attn, np.float32)
    b_attn = np.asarray(b_attn, np.float32)
    w_proj = np.asarray(w_proj, np.float32)
    b_proj = np.asarray(b_proj, np.float32)

    bf16 = ml_dtypes.bfloat16
    xT = np.ascontiguousarray(x.transpose(0, 2, 1)).astype(bf16)  # [B, C, T]
    bqk = np.ascontiguousarray(
        np.concatenate([b_attn[:C] * 0.125, b_attn[C:2 * C]])
        .reshape(NSUB, P).T
    )                                                        # [P, NSUB] fp32
    bv = b_attn[2 * C:].reshape(1, C).astype(bf16)
    bp = b_proj.reshape(1, C).astype(bf16)
    tri = np.triu(np.ones((P, P), bf16))                     # tri[i,j] = j>=i
    # wqk[s, p, kc, c] = w_attn[kc*128+p, s*128+c]; wv2[i, p, kc, c] =
    # w_attn[kc*128+p, 1536+i*384+c] -- contiguous SBUF-layout weight loads
    w4 = w_attn.reshape(KC, P, C3)
    wqk = np.ascontiguousarray(
        w4[:, :, :2 * C].reshape(KC, P, NSUB, P).transpose(2, 1, 0, 3)
    ).astype(bf16)
    wv2 = np.ascontiguousarray(
        w4[:, :, 2 * C:].reshape(KC, P, 2, 384).transpose(2, 1, 0, 3)
    ).astype(bf16)
    shared = {
        "wqk": wqk, "wv2": wv2, "bqk": bqk, "bv": bv, "tri": tri,
        "wp": np.ascontiguousarray(w_proj).astype(bf16), "bp": bp,
    }
    return [dict(shared, xT=np.ascontiguousarray(xT[b])) for b in range(B)]


def kernel(x, w_attn, b_attn, w_proj, b_proj, _run_kwargs=None):
    from concourse.bass_utils import run_bass_kernel_spmd

    with_vbias = bool(np.any(np.asarray(b_attn)[2 * C:]))
    with_pbias = bool(np.any(np.asarray(b_proj)))
    key = ("nc", with_vbias, with_pbias)
    if key not in _CACHED:
        _CACHED[key] = build_program(with_vbias, with_pbias)[0]
    nc = _CACHED[key]
    in_maps = make_in_maps(x, w_attn, b_attn, w_proj, b_proj)
    res = run_bass_kernel_spmd(
        nc, in_maps, core_ids=list(range(N_CORES)), **(_run_kwargs or {})
    )
    out = np.stack([res.results[b]["out"] for b in range(B)]).astype(np.float32)
    if _run_kwargs:
        _CACHED["last_results"] = res
    return out


# revision 41
# speedup vs baseline: 1.0596x; 1.0596x over previous
"""Causal multi-head self-attention block (B=8, T=1024, C=768, H=12) on 8 TRN2
NeuronCores, data-parallel over the batch dimension: core b computes batch b
end-to-end (no collectives).

Layout strategy (per core):
  - host passes x[b] pre-transposed: xT [C=768, T=1024]
  - qT,kT are produced channel-major ([ch, tok]) by using w_attn slices as the
    stationary matmul operand; v is produced token-major ([tok, ch]) by using
    xT slices as stationary.  Both orientations consume the same xT, so the
    kernel needs no on-chip transposes at all.
  - attention scores are computed transposed, attT[k, q] = kT_h.T-free matmul,
    softmax'd as exp(score) without max subtraction (logits here are ~N(0,0.3),
    so exp is safe in fp32), causal-masked via memset + one triangular tile.
  - the softmax denominator is obtained for free by appending a ones column to
    the stationary v operand (65-wide lhsT): row 64 of the AV psum is the sum.
  - y accumulates channel-major (yT), which feeds the output projection with
    w_proj in its native layout; biases are folded in as K=1 matmuls.
"""

from contextlib import ExitStack

import ml_dtypes
import numpy as np

import concourse.bass as bass
import concourse.tile as tile
from concourse import bacc, mybir

N_CORES = 8
B, T, C = 8, 1024, 768
H, HD = 12, 64
C3 = 3 * C
DT = mybir.dt.float32
BF = mybir.dt.bfloat16
AF = mybir.ActivationFunctionType
P = 128
KC = C // P            # 6 k-tiles over the embedding dim
NSUB = (2 * C) // P    # 12 channel blocks covering q and k
TB = T // P            # 8 token blocks
QS = T // 512          # 2 query slices of 512


def _emit(tc: tile.TileContext, io: dict, with_vbias: bool = True, with_pbias: bool = True) -> None:
    nc = tc.nc
    xT_d, wqk_d, wv_d, bqk_d, bv_d, tri_d, wp_d, bp_d, out_d = (
        io["xT"], io["wqk"], io["wv2"], io["bqk"], io["bv"], io["tri"],
        io["wp"], io["bp"], io["out"],
    )

    stack = ExitStack()
    const = stack.enter_context(tc.tile_pool(name="const", bufs=1))
    persist = stack.enter_context(tc.tile_pool(name="persist", bufs=1))

    # ---- constants (tiles; DMAs for non-critical ones are emitted later
    # so the startup burst only carries what the first matmuls need) -------
    tri = const.tile([P, P], BF, tag="tri")          # tri[i,j] = j >= i
    bqk = const.tile([P, NSUB], DT, tag="bqk")       # per-channel qk bias
    nc.sync.dma_start(bqk[:], bqk_d[:, :])
    bv = const.tile([1, C], BF, tag="bv")            # v bias row
    bp = const.tile([1, C], BF, tag="bp")            # proj bias row
    ones = const.tile([1, T], BF, tag="ones")
    nc.vector.memset(ones[:], 1.0)

    # ---- persistent activations -----------------------------------------
    # qkT[s]: channel-major q/k; s 0-5 -> q channels, 6-11 -> k channels
    qkT = [persist.tile([P, T], BF, tag=f"qkT{s}", name=f"qkT{s}") for s in range(NSUB)]
    # v3[tb]: token-major v for token block tb, one 65-wide group per head
    # (64 channels + a ones column used to accumulate softmax denominators)
    v3 = [persist.tile([P, H, HD + 1], BF, tag=f"v{tb}", name=f"v{tb}") for tb in range(TB)]
    # yT[qs][kc]: channel-major attention output, split per q-slice so the
    # qs=0 projection never serializes against qs=1 writes (tile-granular deps)
    yT = [[persist.tile([P, 512], BF, tag=f"yT{q}_{k}", name=f"yT{q}_{k}")
           for k in range(KC)] for q in range(QS)]
    # w_proj, native layout (loaded later, after the hot startup DMAs)
    wp = [persist.tile([P, C], BF, tag=f"wp{k}", name=f"wp{k}") for k in range(KC)]

    # =====================================================================
    # Phase 1: qkv projection
    # =====================================================================
    p1stack = ExitStack()
    ph1 = p1stack.enter_context(tc.tile_pool(name="ph1", bufs=1))
    ph1s = p1stack.enter_context(tc.tile_pool(name="ph1s", bufs=3))
    psum1 = p1stack.enter_context(tc.tile_pool(name="psum1", bufs=2, space="PSUM"))

    # xT split into 512-column halves: the first accumulation chain (i=0)
    # only needs the first-half tiles, so the PE starts ~2x sooner
    xT = [[ph1.tile([P, 512], BF, tag=f"xT{k}_{i}", name=f"xT{k}_{i}")
           for i in range(2)] for k in range(KC)]
    wsubs = [ph1s.tile([P, KC, P], BF, tag="wsub", name=f"wsub{s}", bufs=NSUB)
             for s in range(NSUB)]
    # DMA order = first-consumer order
    nc.sync.dma_start(xT[0][0][:], xT_d[0:P, 0:512])
    nc.sync.dma_start(wsubs[0][:], wqk_d[0])
    for k in range(1, KC):
        nc.sync.dma_start(xT[k][0][:], xT_d[k * P:(k + 1) * P, 0:512])
    for k in range(KC):
        nc.sync.dma_start(xT[k][1][:], xT_d[k * P:(k + 1) * P, 512:1024])
    for s in range(1, NSUB):
        nc.sync.dma_start(wsubs[s][:], wqk_d[s])
    # v-part of w_attn (host-transposed so the load is fully contiguous)
    wv = [ph1.tile([P, KC, 384], BF, tag=f"wv{c}", name=f"wv{c}") for c in range(2)]
    for c in range(2):
        nc.sync.dma_start(wv[c][:], wv_d[c])

    # ones columns of v3 (softmax denominator accumulators)
    for tb in range(TB):
        nc.vector.memset(v3[tb][:, :, HD:HD + 1], 1.0)

    # --- qT / kT: channel-major, w_attn slice stationary -----------------
    for s in range(NSUB):
        wsub = wsubs[s]
        ps = [psum1.tile([P, 512], DT, tag=f"qk{i}", name=f"qk{i}") for i in range(2)]
        for k in range(KC):
            for i in range(2):
                nc.tensor.matmul(
                    ps[i][:], lhsT=wsub[:, k, :], rhs=xT[k][i][:],
                    start=(k == 0), stop=(k == KC - 1),
                )
        # bias add (+ 1/sqrt(hd) folded into q) during psum->sbuf copy,
        # alternating DVE/ACT so neither engine becomes the choke point
        scl = 0.125 if s < KC else 1.0
        for i in range(2):
            dst = qkT[s][:, i * 512:(i + 1) * 512]
            if (2 * s + i) % 2 == 0:
                nc.vector.tensor_scalar(
                    dst, ps[i][:], scl, bqk[:, s:s + 1],
                    mybir.AluOpType.mult, mybir.AluOpType.add,
                )
            else:
                nc.scalar.activation(
                    dst, ps[i][:], AF.Identity, bias=bqk[:, s:s + 1], scale=scl
                )

    # remaining loads, off the startup critical path
    nc.sync.dma_start(tri[:], tri_d[:, :])
    nc.sync.dma_start(bv[:], bv_d[:, :])
    nc.sync.dma_start(bp[:], bp_d[:, :])
    for k in range(KC):
        nc.sync.dma_start(wp[k][:], wp_d[k * P:(k + 1) * P, :])

    # --- v: token-major, xT slice stationary -----------------------------
    for tb in range(TB):
        for c in range(2):
            pv = psum1.tile([P, 384], DT, tag="vps")
            for k in range(KC):
                nc.tensor.matmul(
                    pv[:], lhsT=xT[k][tb // 4][:, (tb % 4) * P:(tb % 4 + 1) * P],
                    rhs=wv[c][:, k, :], start=(k == 0),
                    stop=(not with_vbias and k == KC - 1),
                )
            if with_vbias:
                nc.tensor.matmul(  # + bias via K=1 ones row
                    pv[:], lhsT=ones[:, tb * P:(tb + 1) * P],
                    rhs=bv[:, c * 384:(c + 1) * 384], start=False, stop=True,
                )
            nc.vector.tensor_copy(
                v3[tb][:, c * 6:(c + 1) * 6, 0:HD],
                pv[:].rearrange("p (h f) -> p h f", h=6),
            )

    p1stack.close()

    # =====================================================================
    # Phase 2: attention (scores transposed: [k, q]) + output projection
    # =====================================================================
    p2stack = ExitStack()
    ph2 = p2stack.enter_context(tc.tile_pool(name="ph2", bufs=2))
    psum2_cm = tc.tile_pool(name="psum2", bufs=1, space="PSUM")
    psum2 = psum2_cm.__enter__()
    dram2 = p2stack.enter_context(tc.tile_pool(name="dram2", bufs=2, space="DRAM"))

    def emit_scores_pair(qs, j):
        """score+exp+mask for heads (2j, 2j+1) over all causal k-blocks.

        The two heads live at partition bases 0 and 64 of the same qkT tile,
        so their K=64 matmuls land in different PE row groups and run
        concurrently (tile_position auto-derived from base_partition)."""
        nkb = 4 * (qs + 1)
        ets = ([], [])
        for kb in range(nkb):
            # columns < d are causally dead: matmul/exp/av all skip them
            d = max(kb * P - qs * 512, 0)
            pt = psum2.tile([P, 1024], DT, tag="sps", name="sps", bufs=3)
            for half in range(2):
                po = half * HD
                nc.tensor.matmul(
                    pt[:, half * 512 + d:(half + 1) * 512],
                    lhsT=qkT[KC + j][po:po + HD, kb * P:(kb + 1) * P],
                    rhs=qkT[j][po:po + HD, qs * 512 + d:(qs + 1) * 512],
                    start=True, stop=True,
                )
            # one ACT op exps both heads' valid columns (3D strided AP)
            e = ph2.tile([P, 1024], BF, tag=f"E{kb}", name=f"E{kb}")
            er = e[:].rearrange("p (a f) -> p a f", a=2)[:, :, d:512]
            pr = pt[:].rearrange("p (a f) -> p a f", a=2)[:, :, d:512]
            nc.scalar.activation(er, pr, AF.Exp)
            if kb * P - qs * 512 >= 0:  # diagonal block: triangular mask
                ed = e[:].rearrange("p (a f) -> p a f", a=2)[:, :, d:d + P]
                nc.vector.tensor_tensor(
                    ed, ed, tri[:, None, :].to_broadcast((P, 2, P)),
                    mybir.AluOpType.mult,
                )
            ets[0].append(e)
            ets[1].append(e)
        return ets

    def emit_av_raw(qs, h, etiles, sums_t):
        """unnormalized yT_h[:, qs] = v_ext.T @ E; the fused ones column puts
        the softmax denominator in psum row 64, gathered into sums_t[h]."""
        nkb = len(etiles)
        po, so = (h % 2) * HD, h // 2
        eo = (h % 2) * 512  # this head's half of the fused E tile
        psy = psum2.tile([HD + 1, 512], DT, tag="yps", name="yps", bufs=2)
        for kb in range(nkb):
            d = max(kb * P - qs * 512, 0)
            nc.tensor.matmul(
                psy[:, d:512], lhsT=v3[kb][:, h, :],
                rhs=etiles[kb][:, eo + d:eo + 512],
                start=(kb == 0), stop=(kb == nkb - 1),
            )
        # DVE output bases are limited to 0/32/64/96, so route the sums row
        # into partition h of the gather tile through DRAM (sums_t is DRAM)
        stg = ph2.tile([1, 512], DT, tag="sstage", name="sstage", bufs=3)
        nc.vector.tensor_copy(stg[:], psy[HD:HD + 1, :])
        nc.gpsimd.dma_start(sums_t[h:h + 1, :], stg[:])  # SBUF->SBUF, any partition
        nc.vector.tensor_copy(yT[qs][so][po:po + HD, :], psy[0:HD, :])

    def emit_finish(qs, sums_t):
        """batched reciprocal of all 12 denominators, partition-broadcast via
        a DRAM bounce (SBUF APs need nonzero partition step), normalize yT."""
        rinv = ph2.tile([H, 512], DT, tag="rinv", name="rinv")
        nc.vector.reciprocal(rinv[:], sums_t[:H, :])
        rd = dram2.tile([H, 512], DT, tag="rd", name="rd")
        nc.sync.dma_start(rd[:], rinv[:])
        for j in range(KC):
            # heads 2j/2j+1 -> partitions 0:64/64:128 so every tensor_mul has
            # all SBUF operands at the same start partition (walrus rule)
            rb = ph2.tile([P, 512], DT, tag="rb", name="rb", bufs=3)
            for half in range(2):
                nc.sync.dma_start(
                    rb[half * HD:(half + 1) * HD, :],
                    rd[2 * j + half:2 * j + half + 1, :].to_broadcast((HD, 512)),
                )
            for half in range(2):
                po = half * HD
                ys = yT[qs][j][po:po + HD, :]
                nc.vector.tensor_mul(ys, ys, rb[po:po + HD, :])

    def emit_proj(qs, psum_o):
        for tb in range(4 * qs, 4 * qs + 4):
            for c in range(2):
                pso = psum_o.tile([P, 384], DT, tag="ops", name="ops", bufs=4)
                for k in range(KC):
                    nc.tensor.matmul(
                        pso[:],
                        lhsT=yT[qs][k][:, (tb % 4) * P:(tb % 4 + 1) * P],
                        rhs=wp[k][:, c * 384:(c + 1) * 384],
                        start=(k == 0),
                        stop=(not with_pbias and k == KC - 1),
                    )
                if with_pbias:
                    nc.tensor.matmul(
                        pso[:], lhsT=ones[:, tb * P:(tb + 1) * P],
                        rhs=bp[:, c * 384:(c + 1) * 384], start=False, stop=True,
                    )
                osb = ph2.tile([P, 384], DT, tag="osb", name="osb", bufs=3)
                nc.scalar.activation(osb[:], pso[:], AF.Copy)
                nc.sync.dma_start(
                    out_d[tb * P:(tb + 1) * P, c * 384:(c + 1) * 384], osb[:]
                )

    # Software pipeline at head-pair granularity: scores of pair p+1 are
    # emitted before av of pair p so the PE never waits on the ACT/DVE
    # exp+mask chain; the previous q-slice's normalize+proj is emitted two
    # pairs into the next q-slice so its PE work overlaps attention.
    sums_tiles = {}
    prev = None   # (qs, j, ets)
    fin = None    # q-slice whose normalize+proj is pending
    for qs in range(QS):
        sums_tiles[qs] = ph2.tile([H, 512], DT, tag="sums", name="sums")
        for j in range(KC):
            ets = emit_scores_pair(qs, j)
            if prev is not None:
                pq, pj, pets = prev
                for half in range(2):
                    emit_av_raw(pq, 2 * pj + half, pets[half], sums_tiles[pq])
            prev = (qs, j, ets)
            if fin is not None and j == 3:
                emit_finish(fin, sums_tiles[fin])
                fin = None
        fin = qs
    pq, pj, pets = prev
    for half in range(2):
        emit_av_raw(pq, 2 * pj + half, pets[half], sums_tiles[pq])
    # attention psum pool closes; proj(0) keeps the PE busy while
    # finish(1)'s reciprocal/broadcast chain resolves on DVE/DMA
    psum2_cm.__exit__(None, None, None)
    psum3 = p2stack.enter_context(tc.tile_pool(name="psum3", bufs=1, space="PSUM"))
    emit_finish(1, sums_tiles[1])
    emit_proj(0, psum3)
    emit_proj(1, psum3)

    p2stack.close()
    stack.close()


def build_program(with_vbias: bool = True, with_pbias: bool = True) -> tuple[bass.Bass, dict]:
    nc = bacc.Bacc("TRN2", debug=False)
    io = {
        "xT": nc.dram_tensor("xT", [C, T], BF, kind="ExternalInput"),
        "wqk": nc.dram_tensor("wqk", [NSUB, P, KC, P], BF, kind="ExternalInput"),
        "wv2": nc.dram_tensor("wv2", [2, P, KC, 384], BF, kind="ExternalInput"),
        "bqk": nc.dram_tensor("bqk", [P, NSUB], DT, kind="ExternalInput"),
        "bv": nc.dram_tensor("bv", [1, C], BF, kind="ExternalInput"),
        "tri": nc.dram_tensor("tri", [P, P], BF, kind="ExternalInput"),
        "wp": nc.dram_tensor("wp", [C, C], BF, kind="ExternalInput"),
        "bp": nc.dram_tensor("bp", [1, C], BF, kind="ExternalInput"),
        "out": nc.dram_tensor("out", [T, C], DT, kind="ExternalOutput"),
    }
    with tile.TileContext(nc) as tc:
        _emit(tc, io, with_vbias=with_vbias, with_pbias=with_pbias)
    nc.compile()
    return nc, io


_CACHED = {}


def make_in_maps(x, w_attn, b_attn, w_proj, b_proj):
    x = np.asarray(x, np.float32)
    w_attn = np.asarray(w_attn, np.float32)
    b_attn = np.asarray(b_attn, np.float32)
    w_proj = np.asarray(w_proj, np.float32)
    b_proj = np.asarray(b_proj, np.float32)

    bf16 = ml_dtypes.bfloat16
    xT = np.ascontiguousarray(x.transpose(0, 2, 1)).astype(bf16)  # [B, C, T]
    bqk = np.ascontiguousarray(
        np.concatenate([b_attn[:C] * 0.125, b_attn[C:2 * C]])
        .reshape(NSUB, P).T
    )                                                        # [P, NSUB] fp32
    bv = b_attn[2 * C:].reshape(1, C).astype(bf16)
    bp = b_proj.reshape(1, C).astype(bf16)
    tri = np.triu(np.ones((P, P), bf16))                     # tri[i,j] = j>=i
    # wqk[s, p, kc, c] = w_attn[kc*128+p, s*128+c]; wv2[i, p, kc, c] =
    # w_attn[kc*128+p, 1536+i*384+c] -- contiguous SBUF-layout weight loads
    w4 = w_attn.reshape(KC, P, C3)
    wqk = np.ascontiguousarray(
        w4[:, :, :2 * C].reshape(KC, P, NSUB, P).transpose(2, 1, 0, 3)
    ).astype(bf16)
    wv2 = np.ascontiguousarray(
        w4[:, :, 2 * C:].reshape(KC, P, 2, 384).transpose(2, 1, 0, 3)
    ).astype(bf16)
    shared = {
        "wqk": wqk, "wv2": wv2, "bqk": bqk, "bv": bv, "tri": tri,
        "wp": np.ascontiguousarray(w_proj).astype(bf16), "bp": bp,
    }
    return [dict(shared, xT=np.ascontiguousarray(xT[b])) for b in range(B)]


def kernel(x, w_attn, b_attn, w_proj, b_proj, _run_kwargs=None):
    from concourse.bass_utils import run_bass_kernel_spmd

    with_vbias = bool(np.any(np.asarray(b_attn)[2 * C:]))
    with_pbias = bool(np.any(np.asarray(b_proj)))
    key = ("nc", with_vbias, with_pbias)
    if key not in _CACHED:
        _CACHED[key] = build_program(with_vbias, with_pbias)[0]
    nc = _CACHED[key]
    in_maps = make_in_maps(x, w_attn, b_attn, w_proj, b_proj)
    res = run_bass_kernel_spmd(
        nc, in_maps, core_ids=list(range(N_CORES)), **(_run_kwargs or {})
    )
    out = np.stack([res.results[b]["out"] for b in range(B)]).astype(np.float32)
    if _run_kwargs:
        _CACHED["last_results"] = res
    return out
